# revision 32
# baseline (speedup 1.0000x reference)
"""MoE layer (top-2 of 24 experts, d_model=1024, d_ff=4096, T=4096 tokens)
on 8 Trainium2 NeuronCores.

Strategy (expert-parallel, host-routed):
  - Host computes the gate (x @ gate_w + gate_b), top-2 expert ids and
    softmax probs, then gathers each expert's tokens into a padded,
    transposed buffer xT[e] = [D, C_j].
  - Experts are sharded 3 per core, balanced by token count: experts are
    sorted by count and dealt into 3 "slots" (slot j of every core holds
    one of the j-th-octile experts), so slot capacities C_0 >= C_1 >= C_2
    are exactly the octile maxima (~1089 tokens/core vs the naive
    128-aligned 1280). Both matmul phases stream tokens as the moving
    operand, so compute scales with the actual padded token count.
  - Each core runs a Bass/Tile kernel per slot expert:
      phase A: hT[m] = gelu(w1[e].T-tiled @ xT[e] + b1[e])  [128 dff, C tok]
      phase B: yT[n] = prob * (w2[e]-tiled.T @ hT)          [128 dm,  C tok]
    Phase B keeps tokens as the moving dim (w2 k-tiles are the stationary
    operand); the per-token prob is applied with a DVE tensor_tensor
    multiply against a broadcast prob tile.
  - Weights and x are host-packed so every DMA is contiguous per
    partition (strided multi-segment APs cost 11-29us of HWDGE
    descriptor-generation time); w1 is m-major in 0.5MB sixteenth-chunks
    so the first matmul only waits on ~0.8MB of startup DMA.
  - DMA pipeline: w2[e] streams during phase A of e; w1[e+1] streams
    during phase B of e. All input DMAs ride the sync HWDGE ring in
    priority order (the two rings round-robin the 16 SDMA engines, so a
    split would starve the critical chain); outputs and ACT share the
    scalar ring. Junk matmuls on a memset tile bridge the initial DMA
    wait so the PE's HAM clock gate opens before real work arrives.
  - Host scatters the two per-token expert outputs back together
    (out[t] = y[slot0(t)] + y[slot1(t)]), adds the b2 combine term if b2
    is nonzero (it is zero in this problem's setup_inputs).

Matmuls run in bf16 with fp32 PSUM accumulation (rel err ~3e-3 vs fp32);
b1 is applied exactly on device as the ACT per-partition bias.
"""

import numpy as np
import ml_dtypes

P = 128
D_MODEL = 1024
D_FF = 4096
NUM_EXPERTS = 24
TOP_K = 2
N_CORES = 8
E_LOC = NUM_EXPERTS // N_CORES   # 3 experts per core
KD = D_MODEL // P                # 8  k-chunks over d_model
KF = D_FF // P                   # 32 k-chunks over d_ff
ND2 = D_MODEL // P               # 8  128-wide dm output chunks
ALIGN = 1                        # token-capacity alignment
BF16 = ml_dtypes.bfloat16

W1C = 16                         # w1 macro-chunks (one DMA each)
MPC = KF // W1C                  # 2 m-tiles per w1 chunk
W1CW = MPC * D_MODEL             # 4096 packed columns per w1 macro-tile
W2C = 4                          # w2 macro-chunks (one DMA each)
KPC = KF // W2C                  # 8 k-tiles per w2 chunk
MCW = KPC * D_MODEL              # 8192 packed columns per w2 macro-tile


def _build(Cs, repeat=1):
    """Build the per-core Bass program (SPMD: same program, per-core data).

    Cs: per-slot token capacities (each a multiple of ALIGN).
    repeat: run the whole compute N times (timing rigs only).
    """
    import concourse.bacc as bacc
    import concourse.mybir as mybir
    from concourse.tile import TileContext

    dt = mybir.dt.bfloat16
    f32 = mybir.dt.float32
    CT = sum(Cs)
    offs = [sum(Cs[:j]) for j in range(E_LOC)]

    nc = bacc.Bacc(None, target_bir_lowering=False)
    # xt[p, KD*off_j + k*C_j + c] = x.T[k*128+p, off_j + c] (slot-major,
    # fully contiguous per partition so one DMA per slot issues fast);
    # w1[e, h, p, m4*1024 + k*128 + col] (m-major) packs the [128, 128]
    # lhsT tile for (k-chunk k, m-tile h*4+m4), so the first matmul only
    # waits on one 1MB half-chunk;  w2[e, g, p, k8*1024+col] packs the
    # lhsT tile for (k-chunk g*8+k8, dm-chunk n2) at columns k8*1024 +
    # n2*128.
    xT = nc.dram_tensor("xt", [P, KD * CT], dt, kind="ExternalInput")
    w1 = nc.dram_tensor("w1", [E_LOC, W1C, P, W1CW], dt, kind="ExternalInput")
    w2 = nc.dram_tensor("w2", [E_LOC, W2C, P, MCW], dt, kind="ExternalInput")
    prb = nc.dram_tensor("prb", [P, CT], f32, kind="ExternalInput")
    b1 = nc.dram_tensor("b1", [P, E_LOC * KF], f32, kind="ExternalInput")
    yt = nc.dram_tensor("yt", [ND2, P, CT], f32, kind="ExternalOutput")

    with TileContext(nc) as tc:
        with tc.tile_pool(name="consts", bufs=1) as consts, \
             tc.tile_pool(name="xtp", bufs=E_LOC * KD) as xtp, \
             tc.tile_pool(name="w1p", bufs=W1C + 1) as w1p, \
             tc.tile_pool(name="w2p", bufs=W2C) as w2p, \
             tc.tile_pool(name="htp", bufs=KF + 2) as htp, \
             tc.tile_pool(name="outp", bufs=4) as outp, \
             tc.tile_pool(name="psA", bufs=3, space="PSUM") as psA, \
             tc.tile_pool(name="psB", bufs=4, space="PSUM") as psB, \
             tc.tile_pool(name="psW", bufs=1, space="PSUM") as psW:

            xts = {}   # (slot j, k) -> [P, Cs[j]] tile
            w1ts = {}  # chunk c4 -> [P, 8192] tile, current expert
            w2ts = {}  # chunk g  -> [P, 8192] tile, current expert

            def load_w1_chunk(e, c4):
                t_ = w1p.tile([P, W1CW], dt, tag="w1")
                nc.sync.dma_start(t_[:], w1[e, c4, :, :])
                w1ts[c4] = t_

            def load_w2(e):
                for g in range(W2C):
                    t_ = w2p.tile([P, MCW], dt, tag="w2")
                    nc.sync.dma_start(t_[:], w2[e, g, :, :])
                    w2ts[g] = t_

            def load_xt(j):
                # per-k 2D loads (contiguous slices of the slot-major
                # layout) so the startup chain waits on 104KB, not 830KB
                base = KD * offs[j]
                for k in range(KD):
                    t_ = xtp.tile([P, Cs[j]], dt, tag="xt")
                    nc.sync.dma_start(
                        t_[:], xT[:, base + k * Cs[j]:base + (k + 1) * Cs[j]])
                    xts[(j, k)] = t_

            seq = [e for _ in range(repeat) for e in range(E_LOC)]
            first = True
            for i, e in enumerate(seq):
                C = Cs[e]
                off = offs[e]
                if first:
                    # startup: everything startup-critical goes on the sync
                    # ring in priority order (the two HWDGE rings round-robin
                    # the 16 SDMA engines per packet, so a split would starve
                    # the critical stream): the first w1 chunk (0.5MB) and
                    # slot-0 x k-tiles gate the first real matmul at ~10us;
                    # then the rest of w1, w2 of expert 0 (needed at ~A0
                    # end), and x slots 1,2 (needed at A1). b1 rides the
                    # idle scalar ring; prb (not needed until the first
                    # phase-B DVE) is issued after the first ACT so it does
                    # not steal startup bandwidth.
                    load_xt(0)
                    b1_t = consts.tile([P, E_LOC * KF], f32, tag="b1")
                    nc.scalar.dma_start(b1_t[:], b1[:, :])
                    prb_t = consts.tile([P, CT], f32, tag="prb")
                    for c4 in range(W1C):
                        load_w1_chunk(e, c4)
                    load_w2(e)
                    for j in range(1, E_LOC):
                        load_xt(j)
                    # PE warmup: junk matmuls on a zero-memset tile (no DMA
                    # dependency) keep the PE busy from ~7.5us so the HAM
                    # clock gate opens (1.2->2.4GHz) soon after the first
                    # real matmul; the result is never read.
                    jt = consts.tile([P, 512], dt, tag="junk")
                    nc.vector.memset(jt[:], 0)
                    ps_w = psW.tile([P, 512], f32, tag="psW")
                    for _ in range(8):
                        nc.tensor.matmul(ps_w[:], jt[:, :P], jt[:],
                                         start=True, stop=True)
                    first = False

                NT = (C + 511) // 512
                hts_all = []
                for t in range(NT):
                    c0 = t * 512
                    W = min(512, C - c0)
                    # phase A: hT[m] = gelu(w1k.T @ xT + b1), [P dff x W tok]
                    hts = []
                    for m in range(KF):
                        ps = psA.tile([P, W], f32, tag="psA")
                        w1t = w1ts[m // MPC]
                        mc = (m % MPC) * D_MODEL
                        for k in range(KD):
                            nc.tensor.matmul(
                                ps[:],
                                w1t[:, mc + k * P:mc + k * P + P],
                                xts[(e, k)][:, c0:c0 + W],
                                start=(k == 0), stop=(k == KD - 1))
                        ht = htp.tile([P, W], dt, tag="ht")
                        nc.scalar.activation(
                            ht[:], ps[:],
                            mybir.ActivationFunctionType.Gelu,
                            bias=b1_t[:, e * KF + m: e * KF + m + 1])
                        hts.append(ht)
                        if i == 0 and t == 0 and m == 0:
                            # deferred prb load (see startup comment)
                            nc.scalar.dma_start(prb_t[:], prb[:, :])
                    hts_all.append(hts)

                # prefetch next expert's w1 (streams during phase B below)
                if i + 1 < len(seq):
                    for c4 in range(W1C):
                        load_w1_chunk(seq[i + 1], c4)

                # phase B: yT[n2] = prob * (w2k-chunk.T @ hT), [P dm x W tok]
                w2_cur = dict(w2ts)
                for t in range(NT):
                    c0 = t * 512
                    W = min(512, C - c0)
                    hts = hts_all[t]
                    for n2 in range(ND2):
                        ps = psB.tile([P, W], f32, tag="psB")
                        for k in range(KF):
                            nc.tensor.matmul(
                                ps[:],
                                w2_cur[k // KPC][:, (k % KPC) * D_MODEL
                                                 + n2 * P:
                                                 (k % KPC) * D_MODEL
                                                 + n2 * P + P],
                                hts[k][:],
                                start=(k == 0), stop=(k == KF - 1))
                        ot = outp.tile([P, W], f32, tag="out")
                        nc.vector.tensor_mul(
                            ot[:], ps[:], prb_t[:, off + c0:off + c0 + W])
                        nc.scalar.dma_start(
                            yt[n2, :, off + c0:off + c0 + W], ot[:])

                # next expert's w2 (streams during its phase A)
                if i + 1 < len(seq):
                    load_w2(seq[i + 1])
    nc.finalize()
    return nc


def _route(x, gate_w, gate_b):
    """Top-2 routing on host. Returns flattened (expert, prob) per routed
    pair, the by-expert sort order, per-expert counts/starts, and each
    pair's position within its expert segment."""
    T = x.shape[0]
    scores = x @ gate_w + gate_b                      # [T, E]
    part = np.argpartition(scores, -TOP_K, axis=1)[:, -TOP_K:]   # [T, 2]
    vals = np.take_along_axis(scores, part, axis=1)
    vmax = vals.max(axis=1, keepdims=True)
    ex = np.exp(vals - vmax)
    prob = ex / ex.sum(axis=1, keepdims=True)

    expert_flat = part.ravel()                        # [2T]
    prob_flat = prob.ravel().astype(np.float32)
    token_flat = np.repeat(np.arange(T), TOP_K)

    order = np.argsort(expert_flat, kind="stable")
    counts = np.bincount(expert_flat, minlength=NUM_EXPERTS)
    starts = np.zeros(NUM_EXPERTS + 1, dtype=np.int64)
    np.cumsum(counts, out=starts[1:])

    inv_order = np.empty_like(order)
    inv_order[order] = np.arange(order.size)
    pos = inv_order - starts[expert_flat]
    return (expert_flat, prob_flat, token_flat, order, counts, starts, pos)


def _prepare(x, gate_w, gate_b, w1, b1, w2, b2):
    """Host-side routing, balanced expert->(core,slot) assignment, and
    per-core input packing. Returns (in_maps, Cs, meta-for-combine)."""
    B, S, D = x.shape
    T = B * S
    xf = np.ascontiguousarray(x.reshape(T, D), dtype=np.float32)

    (expert_flat, prob_flat, token_flat, order, counts, starts, pos) = _route(
        xf, np.asarray(gate_w, np.float32), np.asarray(gate_b, np.float32))

    # balanced assignment: slot j of core c holds expert_desc[j*8 + c]
    expert_desc = np.argsort(-counts, kind="stable")
    core_of = np.empty(NUM_EXPERTS, dtype=np.int64)
    slot_of = np.empty(NUM_EXPERTS, dtype=np.int64)
    for j in range(E_LOC):
        for c in range(N_CORES):
            e = expert_desc[j * N_CORES + c]
            core_of[e] = c
            slot_of[e] = j
    Cs = []
    for j in range(E_LOC):
        mx = counts[expert_desc[j * N_CORES:(j + 1) * N_CORES]].max()
        Cs.append(max(ALIGN, int(-(-int(mx) // ALIGN)) * ALIGN))
    CT = sum(Cs)
    offs = [sum(Cs[:j]) for j in range(E_LOC)]

    xg16 = xf[token_flat[order]].astype(BF16)         # [2T, D] sorted by expert
    sorted_probs = prob_flat[order]

    w1_16 = np.asarray(w1, np.float32).astype(BF16)   # [E, D, F]
    w2_16 = np.asarray(w2, np.float32).astype(BF16)   # [E, F, D]
    b1_f = np.asarray(b1, np.float32)                 # [E, F]

    in_maps = []
    for c in range(N_CORES):
        xt_core = np.zeros((D, CT), dtype=BF16)
        pr_core = np.zeros((CT,), dtype=np.float32)
        w1_core = np.empty((E_LOC, D, D_FF), dtype=BF16)
        w2_core = np.empty((E_LOC, D_FF, D), dtype=BF16)
        b1_core = np.empty((E_LOC, D_FF), dtype=np.float32)
        for j in range(E_LOC):
            e = expert_desc[j * N_CORES + c]
            c_e = counts[e]
            if c_e:
                seg = slice(starts[e], starts[e] + c_e)
                xt_core[:, offs[j]:offs[j] + c_e] = xg16[seg].T
                pr_core[offs[j]:offs[j] + c_e] = sorted_probs[seg]
            w1_core[j] = w1_16[e]
            w2_core[j] = w2_16[e]
            b1_core[j] = b1_f[e]
        # pack layouts (see _build dram tensor comments)
        xt_pack = np.empty((P, KD * CT), dtype=BF16)
        for j in range(E_LOC):
            o, Cj = offs[j], Cs[j]
            xt_pack[:, KD * o:KD * (o + Cj)] = (
                xt_core[:, o:o + Cj].reshape(KD, P, Cj)
                .transpose(1, 0, 2).reshape(P, KD * Cj))
        w1_pack = (w1_core.reshape(E_LOC, KD, P, W1C, MPC, P)
                   .transpose(0, 3, 2, 4, 1, 5).reshape(E_LOC, W1C, P, W1CW))
        w2_pack = (w2_core.reshape(E_LOC, W2C, KPC, P, D)
                   .transpose(0, 1, 3, 2, 4).reshape(E_LOC, W2C, P, MCW))
        in_maps.append({
            "xt": np.ascontiguousarray(xt_pack),
            "w1": np.ascontiguousarray(w1_pack),
            "w2": np.ascontiguousarray(w2_pack),
            "prb": np.ascontiguousarray(
                np.broadcast_to(pr_core[None, :], (P, CT))),
            "b1": np.ascontiguousarray(
                b1_core.reshape(E_LOC, KF, P).transpose(2, 0, 1)
                .reshape(P, E_LOC * KF)),
        })

    meta = dict(T=T, shape=x.shape, CT=CT, offs=offs,
                core_of=core_of, slot_of=slot_of,
                expert_flat=expert_flat, prob_flat=prob_flat,
                token_flat=token_flat, pos=pos, b2=np.asarray(b2, np.float32))
    return in_maps, Cs, meta


def _unpack_y(res_core):
    """Device output yt [ND2, P, CT] -> y [CT, D]."""
    yt = res_core["yt"]
    return yt.transpose(2, 0, 1).reshape(yt.shape[2], D_MODEL)


def _combine(y_per_core, meta):
    """out[t] = sum of the token's two routed expert outputs (+ b2 term)."""
    T = meta["T"]
    CT = meta["CT"]
    offs = np.asarray(meta["offs"], dtype=np.int64)
    expert_flat = meta["expert_flat"]
    y_flat = np.concatenate(y_per_core, axis=0)       # [8*CT, D]

    rows = (meta["core_of"][expert_flat] * CT
            + offs[meta["slot_of"][expert_flat]] + meta["pos"])
    rows = rows.reshape(T, TOP_K)
    out = y_flat[rows[:, 0]] + y_flat[rows[:, 1]]

    b2_f = meta["b2"]
    if np.any(b2_f):
        combine = np.zeros((T, NUM_EXPERTS), dtype=np.float32)
        np.add.at(combine, (meta["token_flat"], expert_flat), meta["prob_flat"])
        out += combine @ b2_f
    return np.ascontiguousarray(out.reshape(meta["shape"]), dtype=np.float32)


def kernel(x, gate_w, gate_b, w1, b1, w2, b2):
    from concourse import bass_utils

    in_maps, Cs, meta = _prepare(x, gate_w, gate_b, w1, b1, w2, b2)
    nc = _build(Cs)
    res = bass_utils.run_bass_kernel_spmd(nc, in_maps, core_ids=list(range(N_CORES)))
    return _combine([_unpack_y(res.results[c]) for c in range(N_CORES)], meta)


# revision 35
# speedup vs baseline: 1.0065x; 1.0065x over previous
"""MoE layer (top-2 of 24 experts, d_model=1024, d_ff=4096, T=4096 tokens)
on 8 Trainium2 NeuronCores.

Strategy (expert-parallel, host-routed):
  - Host computes the gate (x @ gate_w + gate_b), top-2 expert ids and
    softmax probs, then gathers each expert's tokens into a padded,
    transposed buffer xT[e] = [D, C_j].
  - Experts are sharded 3 per core, balanced by token count: experts are
    sorted by count and dealt into 3 "slots" (slot j of every core holds
    one of the j-th-octile experts), so slot capacities C_0 >= C_1 >= C_2
    are exactly the octile maxima (~1089 tokens/core vs the naive
    128-aligned 1280). Both matmul phases stream tokens as the moving
    operand, so compute scales with the actual padded token count.
  - Each core runs a Bass/Tile kernel per slot expert:
      phase A: hT[m] = gelu(w1[e].T-tiled @ xT[e] + b1[e])  [128 dff, C tok]
      phase B: yT[n] = prob * (w2[e]-tiled.T @ hT)          [128 dm,  C tok]
    Phase B keeps tokens as the moving dim (w2 k-tiles are the stationary
    operand); the per-token prob is applied with a DVE tensor_tensor
    multiply against a broadcast prob tile.
  - Weights and x are host-packed so every DMA is contiguous per
    partition (strided multi-segment APs cost 11-29us of HWDGE
    descriptor-generation time); w1 is m-major in 0.5MB sixteenth-chunks
    so the first matmul only waits on ~0.8MB of startup DMA.
  - DMA pipeline: w2[e] streams during phase A of e; w1[e+1] streams
    during phase B of e. All input DMAs ride the sync HWDGE ring in
    priority order (the two rings round-robin the 16 SDMA engines, so a
    split would starve the critical chain); outputs and ACT share the
    scalar ring. Junk matmuls on a memset tile bridge the initial DMA
    wait so the PE's HAM clock gate opens before real work arrives.
  - Host scatters the two per-token expert outputs back together
    (out[t] = y[slot0(t)] + y[slot1(t)]), adds the b2 combine term if b2
    is nonzero (it is zero in this problem's setup_inputs).

Matmuls run in bf16 with fp32 PSUM accumulation (rel err ~3e-3 vs fp32);
b1 is applied exactly on device as the ACT per-partition bias.
"""

import numpy as np
import ml_dtypes

P = 128
D_MODEL = 1024
D_FF = 4096
NUM_EXPERTS = 24
TOP_K = 2
N_CORES = 8
E_LOC = NUM_EXPERTS // N_CORES   # 3 experts per core
KD = D_MODEL // P                # 8  k-chunks over d_model
KF = D_FF // P                   # 32 k-chunks over d_ff
ND2 = D_MODEL // P               # 8  128-wide dm output chunks
ALIGN = 1                        # token-capacity alignment
BF16 = ml_dtypes.bfloat16

W1C = 16                         # w1 macro-chunks (one DMA each)
MPC = KF // W1C                  # 2 m-tiles per w1 chunk
W1CW = MPC * D_MODEL             # 4096 packed columns per w1 macro-tile
W2C = 4                          # w2 macro-chunks (one DMA each)
KPC = KF // W2C                  # 8 k-tiles per w2 chunk
MCW = KPC * D_MODEL              # 8192 packed columns per w2 macro-tile


def _build(Cs, repeat=1):
    """Build the per-core Bass program (SPMD: same program, per-core data).

    Cs: per-slot token capacities (each a multiple of ALIGN).
    repeat: run the whole compute N times (timing rigs only).
    """
    import concourse.bacc as bacc
    import concourse.mybir as mybir
    from concourse.tile import TileContext

    dt = mybir.dt.bfloat16
    f32 = mybir.dt.float32
    CT = sum(Cs)
    offs = [sum(Cs[:j]) for j in range(E_LOC)]

    nc = bacc.Bacc(None, target_bir_lowering=False)
    # xt[p, KD*off_j + k*C_j + c] = x.T[k*128+p, off_j + c] (slot-major,
    # fully contiguous per partition so one DMA per slot issues fast);
    # w1[e, h, p, m4*1024 + k*128 + col] (m-major) packs the [128, 128]
    # lhsT tile for (k-chunk k, m-tile h*4+m4), so the first matmul only
    # waits on one 1MB half-chunk;  w2[e, g, p, k8*1024+col] packs the
    # lhsT tile for (k-chunk g*8+k8, dm-chunk n2) at columns k8*1024 +
    # n2*128.
    xT = nc.dram_tensor("xt", [P, KD * CT], dt, kind="ExternalInput")
    w1 = nc.dram_tensor("w1", [E_LOC, W1C, P, W1CW], dt, kind="ExternalInput")
    w2 = nc.dram_tensor("w2", [E_LOC, W2C, P, MCW], dt, kind="ExternalInput")
    prb = nc.dram_tensor("prb", [P, CT], f32, kind="ExternalInput")
    b1 = nc.dram_tensor("b1", [P, E_LOC * KF], f32, kind="ExternalInput")
    yt = nc.dram_tensor("yt", [ND2, P, CT], f32, kind="ExternalOutput")

    with TileContext(nc) as tc:
        with tc.tile_pool(name="consts", bufs=1) as consts, \
             tc.tile_pool(name="xtp", bufs=E_LOC * KD) as xtp, \
             tc.tile_pool(name="w1p", bufs=W1C + 1) as w1p, \
             tc.tile_pool(name="w2p", bufs=W2C) as w2p, \
             tc.tile_pool(name="htp", bufs=KF + 2) as htp, \
             tc.tile_pool(name="outp", bufs=4) as outp, \
             tc.tile_pool(name="psA", bufs=3, space="PSUM") as psA, \
             tc.tile_pool(name="psB", bufs=4, space="PSUM") as psB, \
             tc.tile_pool(name="psW", bufs=1, space="PSUM") as psW:

            xts = {}   # (slot j, k) -> [P, Cs[j]] tile
            w1ts = {}  # chunk c4 -> [P, 8192] tile, current expert
            w2ts = {}  # chunk g  -> [P, 8192] tile, current expert

            def load_w1_chunk(e, c4):
                t_ = w1p.tile([P, W1CW], dt, tag="w1")
                nc.sync.dma_start(t_[:], w1[e, c4, :, :])
                w1ts[c4] = t_

            def load_w2(e):
                for g in range(W2C):
                    t_ = w2p.tile([P, MCW], dt, tag="w2")
                    nc.sync.dma_start(t_[:], w2[e, g, :, :])
                    w2ts[g] = t_

            def load_xt(j):
                # per-k 2D loads (contiguous slices of the slot-major
                # layout) so the startup chain waits on 104KB, not 830KB
                base = KD * offs[j]
                for k in range(KD):
                    t_ = xtp.tile([P, Cs[j]], dt, tag="xt")
                    nc.sync.dma_start(
                        t_[:], xT[:, base + k * Cs[j]:base + (k + 1) * Cs[j]])
                    xts[(j, k)] = t_

            seq = [e for _ in range(repeat) for e in range(E_LOC)]
            first = True
            for i, e in enumerate(seq):
                C = Cs[e]
                off = offs[e]
                if first:
                    # startup: everything startup-critical goes on the sync
                    # ring in priority order (the two HWDGE rings round-robin
                    # the 16 SDMA engines per packet, so a split would starve
                    # the critical stream): the first w1 chunk (0.5MB) and
                    # slot-0 x k-tiles gate the first real matmul at ~10us;
                    # then the rest of w1, w2 of expert 0 (needed at ~A0
                    # end), and x slots 1,2 (needed at A1). b1 rides the
                    # idle scalar ring; prb (not needed until the first
                    # phase-B DVE) is issued after the first ACT so it does
                    # not steal startup bandwidth.
                    load_w1_chunk(e, 0)
                    b1_t = consts.tile([P, E_LOC * KF], f32, tag="b1")
                    nc.scalar.dma_start(b1_t[:], b1[:, :])
                    prb_t = consts.tile([P, CT], f32, tag="prb")
                    load_xt(0)
                    for c4 in range(1, W1C):
                        load_w1_chunk(e, c4)
                    load_w2(e)
                    for j in range(1, E_LOC):
                        load_xt(j)
                    # PE warmup: junk matmuls on a zero-memset tile (no DMA
                    # dependency) keep the PE busy from ~7.5us so the HAM
                    # clock gate opens (1.2->2.4GHz) soon after the first
                    # real matmul; the result is never read.
                    jt = consts.tile([P, 512], dt, tag="junk")
                    nc.vector.memset(jt[:], 0)
                    ps_w = psW.tile([P, 512], f32, tag="psW")
                    for _ in range(6):
                        nc.tensor.matmul(ps_w[:], jt[:, :P], jt[:],
                                         start=True, stop=True)
                    first = False

                NT = (C + 511) // 512
                hts_all = []
                for t in range(NT):
                    c0 = t * 512
                    W = min(512, C - c0)
                    # phase A: hT[m] = gelu(w1k.T @ xT + b1), [P dff x W tok]
                    hts = []
                    for m in range(KF):
                        ps = psA.tile([P, W], f32, tag="psA")
                        w1t = w1ts[m // MPC]
                        mc = (m % MPC) * D_MODEL
                        for k in range(KD):
                            nc.tensor.matmul(
                                ps[:],
                                w1t[:, mc + k * P:mc + k * P + P],
                                xts[(e, k)][:, c0:c0 + W],
                                start=(k == 0), stop=(k == KD - 1))
                        ht = htp.tile([P, W], dt, tag="ht")
                        nc.scalar.activation(
                            ht[:], ps[:],
                            mybir.ActivationFunctionType.Gelu,
                            bias=b1_t[:, e * KF + m: e * KF + m + 1])
                        hts.append(ht)
                        if i == 0 and t == 0 and m == 0:
                            # deferred prb load (see startup comment)
                            nc.scalar.dma_start(prb_t[:], prb[:, :])
                    hts_all.append(hts)

                # prefetch next expert's w1 (streams during phase B below)
                if i + 1 < len(seq):
                    for c4 in range(W1C):
                        load_w1_chunk(seq[i + 1], c4)

                # phase B: yT[n2] = prob * (w2k-chunk.T @ hT), [P dm x W tok]
                w2_cur = dict(w2ts)
                for t in range(NT):
                    c0 = t * 512
                    W = min(512, C - c0)
                    hts = hts_all[t]
                    last_n2 = (i == len(seq) - 1 and t == NT - 1)
                    for n2 in range(ND2):
                        # the kernel's very last psB is processed in two
                        # token-halves so the final DVE+DMA+HBM-receipt
                        # chain (the serial tail after the last matmul) is
                        # half-sized, with the first half draining during
                        # the second half's matmuls.
                        if last_n2 and n2 == ND2 - 1 and W > 128:
                            halves = [(0, W - W // 4), (W - W // 4, W)]
                        else:
                            halves = [(0, W)]
                        for h0, h1 in halves:
                            ps = psB.tile([P, h1 - h0], f32, tag="psB")
                            for k in range(KF):
                                nc.tensor.matmul(
                                    ps[:],
                                    w2_cur[k // KPC][:, (k % KPC) * D_MODEL
                                                     + n2 * P:
                                                     (k % KPC) * D_MODEL
                                                     + n2 * P + P],
                                    hts[k][:, h0:h1],
                                    start=(k == 0), stop=(k == KF - 1))
                            ot = outp.tile([P, h1 - h0], f32, tag="out")
                            nc.vector.tensor_mul(
                                ot[:], ps[:],
                                prb_t[:, off + c0 + h0:off + c0 + h1])
                            nc.scalar.dma_start(
                                yt[n2, :, off + c0 + h0:off + c0 + h1],
                                ot[:])

                # next expert's w2 (streams during its phase A)
                if i + 1 < len(seq):
                    load_w2(seq[i + 1])
    nc.finalize()
    return nc


def _route(x, gate_w, gate_b):
    """Top-2 routing on host. Returns flattened (expert, prob) per routed
    pair, the by-expert sort order, per-expert counts/starts, and each
    pair's position within its expert segment."""
    T = x.shape[0]
    scores = x @ gate_w + gate_b                      # [T, E]
    part = np.argpartition(scores, -TOP_K, axis=1)[:, -TOP_K:]   # [T, 2]
    vals = np.take_along_axis(scores, part, axis=1)
    vmax = vals.max(axis=1, keepdims=True)
    ex = np.exp(vals - vmax)
    prob = ex / ex.sum(axis=1, keepdims=True)

    expert_flat = part.ravel()                        # [2T]
    prob_flat = prob.ravel().astype(np.float32)
    token_flat = np.repeat(np.arange(T), TOP_K)

    order = np.argsort(expert_flat, kind="stable")
    counts = np.bincount(expert_flat, minlength=NUM_EXPERTS)
    starts = np.zeros(NUM_EXPERTS + 1, dtype=np.int64)
    np.cumsum(counts, out=starts[1:])

    inv_order = np.empty_like(order)
    inv_order[order] = np.arange(order.size)
    pos = inv_order - starts[expert_flat]
    return (expert_flat, prob_flat, token_flat, order, counts, starts, pos)


def _prepare(x, gate_w, gate_b, w1, b1, w2, b2):
    """Host-side routing, balanced expert->(core,slot) assignment, and
    per-core input packing. Returns (in_maps, Cs, meta-for-combine)."""
    B, S, D = x.shape
    T = B * S
    xf = np.ascontiguousarray(x.reshape(T, D), dtype=np.float32)

    (expert_flat, prob_flat, token_flat, order, counts, starts, pos) = _route(
        xf, np.asarray(gate_w, np.float32), np.asarray(gate_b, np.float32))

    # balanced assignment: slot j of core c holds expert_desc[j*8 + c]
    expert_desc = np.argsort(-counts, kind="stable")
    core_of = np.empty(NUM_EXPERTS, dtype=np.int64)
    slot_of = np.empty(NUM_EXPERTS, dtype=np.int64)
    for j in range(E_LOC):
        for c in range(N_CORES):
            e = expert_desc[j * N_CORES + c]
            core_of[e] = c
            slot_of[e] = j
    Cs = []
    for j in range(E_LOC):
        mx = counts[expert_desc[j * N_CORES:(j + 1) * N_CORES]].max()
        Cs.append(max(ALIGN, int(-(-int(mx) // ALIGN)) * ALIGN))
    CT = sum(Cs)
    offs = [sum(Cs[:j]) for j in range(E_LOC)]

    xg16 = xf[token_flat[order]].astype(BF16)         # [2T, D] sorted by expert
    sorted_probs = prob_flat[order]

    w1_16 = np.asarray(w1, np.float32).astype(BF16)   # [E, D, F]
    w2_16 = np.asarray(w2, np.float32).astype(BF16)   # [E, F, D]
    b1_f = np.asarray(b1, np.float32)                 # [E, F]

    in_maps = []
    for c in range(N_CORES):
        xt_core = np.zeros((D, CT), dtype=BF16)
        pr_core = np.zeros((CT,), dtype=np.float32)
        w1_core = np.empty((E_LOC, D, D_FF), dtype=BF16)
        w2_core = np.empty((E_LOC, D_FF, D), dtype=BF16)
        b1_core = np.empty((E_LOC, D_FF), dtype=np.float32)
        for j in range(E_LOC):
            e = expert_desc[j * N_CORES + c]
            c_e = counts[e]
            if c_e:
                seg = slice(starts[e], starts[e] + c_e)
                xt_core[:, offs[j]:offs[j] + c_e] = xg16[seg].T
                pr_core[offs[j]:offs[j] + c_e] = sorted_probs[seg]
            w1_core[j] = w1_16[e]
            w2_core[j] = w2_16[e]
            b1_core[j] = b1_f[e]
        # pack layouts (see _build dram tensor comments)
        xt_pack = np.empty((P, KD * CT), dtype=BF16)
        for j in range(E_LOC):
            o, Cj = offs[j], Cs[j]
            xt_pack[:, KD * o:KD * (o + Cj)] = (
                xt_core[:, o:o + Cj].reshape(KD, P, Cj)
                .transpose(1, 0, 2).reshape(P, KD * Cj))
        w1_pack = (w1_core.reshape(E_LOC, KD, P, W1C, MPC, P)
                   .transpose(0, 3, 2, 4, 1, 5).reshape(E_LOC, W1C, P, W1CW))
        w2_pack = (w2_core.reshape(E_LOC, W2C, KPC, P, D)
                   .transpose(0, 1, 3, 2, 4).reshape(E_LOC, W2C, P, MCW))
        in_maps.append({
            "xt": np.ascontiguousarray(xt_pack),
            "w1": np.ascontiguousarray(w1_pack),
            "w2": np.ascontiguousarray(w2_pack),
            "prb": np.ascontiguousarray(
                np.broadcast_to(pr_core[None, :], (P, CT))),
            "b1": np.ascontiguousarray(
                b1_core.reshape(E_LOC, KF, P).transpose(2, 0, 1)
                .reshape(P, E_LOC * KF)),
        })

    meta = dict(T=T, shape=x.shape, CT=CT, offs=offs,
                core_of=core_of, slot_of=slot_of,
                expert_flat=expert_flat, prob_flat=prob_flat,
                token_flat=token_flat, pos=pos, b2=np.asarray(b2, np.float32))
    return in_maps, Cs, meta


def _unpack_y(res_core):
    """Device output yt [ND2, P, CT] -> y [CT, D]."""
    yt = res_core["yt"]
    return yt.transpose(2, 0, 1).reshape(yt.shape[2], D_MODEL)


def _combine(y_per_core, meta):
    """out[t] = sum of the token's two routed expert outputs (+ b2 term)."""
    T = meta["T"]
    CT = meta["CT"]
    offs = np.asarray(meta["offs"], dtype=np.int64)
    expert_flat = meta["expert_flat"]
    y_flat = np.concatenate(y_per_core, axis=0)       # [8*CT, D]

    rows = (meta["core_of"][expert_flat] * CT
            + offs[meta["slot_of"][expert_flat]] + meta["pos"])
    rows = rows.reshape(T, TOP_K)
    out = y_flat[rows[:, 0]] + y_flat[rows[:, 1]]

    b2_f = meta["b2"]
    if np.any(b2_f):
        combine = np.zeros((T, NUM_EXPERTS), dtype=np.float32)
        np.add.at(combine, (meta["token_flat"], expert_flat), meta["prob_flat"])
        out += combine @ b2_f
    return np.ascontiguousarray(out.reshape(meta["shape"]), dtype=np.float32)


def kernel(x, gate_w, gate_b, w1, b1, w2, b2):
    from concourse import bass_utils

    in_maps, Cs, meta = _prepare(x, gate_w, gate_b, w1, b1, w2, b2)
    nc = _build(Cs)
    res = bass_utils.run_bass_kernel_spmd(nc, in_maps, core_ids=list(range(N_CORES)))
    return _combine([_unpack_y(res.results[c]) for c in range(N_CORES)], meta)
